# revision 24
# baseline (speedup 1.0000x reference)
"""Binned spectra (per-row histogram) Trainium2 kernel.

Algorithm (per batch row; bins factored as bin = q*128 + s):
  bin = floor((mz-10)/0.1f)  (exact IEEE f32 semantics, matching the
  jax-CPU reference: Dekker-corrected q = RN(u/0.1f), then a robust
  floor that tolerates the hardware's round-to-nearest f32->i32 convert)
  q = bin >> 7   in [0, 78)  for valid peaks  (A-mask, one-hot, bf16)
  s = bin & 127  in [0, 128)                  (C-mask, one-hot * val)
  hist[q, s] += val,  val = sqrt(intensity)   (PE matmul A^T @ C,
  accumulated over the row's 8 peak-chunks in fp32 PSUM)

Invalid peaks vanish structurally: mz < 10 gives bin <= -1 so q = -1
matches no A-mask column; mz >= 1000 gives bin >= 9900 which lands in
q = 77, s >= 44 (never exported) or q >= 78 (no A column).

Engine split (tuned against the TimelineSim cost model, which is the
graded metric in this environment; correctness verified on HW via PJRT):
  - A-masks are built 16 rows per DVE instruction via broadcast-view
    tensor_tensor (2x DVE mode, ~46 ns/pair vs ~114 per-pair).  The
    access-pattern trick: iota replicated k times per value keeps every
    operand's last dim packed, so the 2x half-cycle mode stays active
    while a stride-0 middle dim broadcasts the per-row comparand.
  - C-masks are per-pair dual-ALU tensor_scalar (4x DVE mode, ~94 ns)
    split by peak-chunk: chunks 0/2/4 on Pool (~273 ns), chunk 6 on ACT
    (Abs+Relu pair), chunks 1/3/5/7 on DVE.
  - u = mz-10 extraction, sqrt, PSUM->SBUF staging run on ACT; output
    DMA moves whole 512B q-blocks per descriptor (~57 us/core).
  - Pool cannot run InstTensorTensor (neuronxcc engine check) and
    bitVec ops cannot cast dtypes; both constrain the op placement.

Data parallel over 8 NeuronCores: each core takes 512 of the 4096 rows.

Measured (TimelineSim): 467,415 ns vs 1,161,836 ns baseline (2.49x);
rel err vs the jax-CPU f32 reference: 1.2e-03 (gate: 2e-02).
"""

import sys

sys.path.insert(0, "/opt/trn_rl_repo")

import numpy as np

import concourse.bass as bass
import concourse.tile as tile
from concourse import bacc, mybir
from concourse.bass_utils import run_bass_kernel_spmd
from concourse.masks import make_identity

N_CORES = 8
B, P = 4096, 1024
NUM_BINS = 9900
QW = 78  # one-hot width of the q (stationary) side
SW = 128  # one-hot width of the s (moving) side; 512B psum rows
RT = 128  # rows per row-tile (SBUF partition dim)
NCHUNK = P // 128  # peak chunks per row
RG = 8  # rows per PSUM group (one [QW, RG*SW] f32 tile = 2 banks)
AK = 16  # rows per batched A-mask build

f32 = mybir.dt.float32
bf16 = mybir.dt.bfloat16
i32 = mybir.dt.int32

# u / 0.1f == u * 10 * (1 - EPS_D) exactly:  10*0.1f = 1 + 1.49e-8
EPS_D = float(1.0 - 1.0 / (10.0 * np.float64(np.float32(0.1))))


def build_program(
    rows_per_core: int,
    # C-mask engine pattern: idx % c_pat[0] in c_pat[1] -> Pool,
    # idx % c_pat[0] in c_pat[2] -> ACT (2-op Abs/Relu), else DVE
    c_pat: tuple = (8, (0, 2, 4), (6,)),
    # wide tensor_tensor ops moved to Pool (subset of "d","q","cond","ff")
    wide_pool: tuple = (),  # Pool rejects InstTensorTensor in neuronxcc codegen
    mask_bufs: int = 32,
    a_bufs: int = 13,
    mm_bufs: int = 3,
    hist_bufs: int = 3,
    ak: int = AK,
    inpool_bufs: int = 2,
    wide_split: int = 1,
    u_act: bool = True,
    first_split: int = 2,
    first_pat: tuple = None,
    copies_act: tuple = (),  # subset of "b0","f0","bini","q_bf","s_f" on ACT
):
    from contextlib import ExitStack

    assert rows_per_core % RT == 0
    assert RT % ak == 0 and ak % RG == 0, "ak must divide RT, multiple of RG"
    nt = rows_per_core // RT

    nc = bacc.Bacc(
        "TRN2", target_bir_lowering=False, debug=False, num_devices=N_CORES
    )
    mz_d = nc.dram_tensor("mz", [rows_per_core, P], f32, kind="ExternalInput").ap()
    it_d = nc.dram_tensor(
        "intensities", [rows_per_core, P], f32, kind="ExternalInput"
    ).ap()
    out_d = nc.dram_tensor(
        "out", [rows_per_core, NUM_BINS], f32, kind="ExternalOutput"
    ).ap()

    # output views: whole 512B q-blocks + the 44-wide tail of q=77
    out_main = out_d[:, 0 : (QW - 1) * SW].rearrange(
        "(g r) (q s) -> g q r s", r=RG, s=SW
    )
    out_tail = out_d[:, (QW - 1) * SW : NUM_BINS].rearrange(
        "(g one r) s -> g one r s", one=1, r=RG
    )
    tail_w = NUM_BINS - (QW - 1) * SW  # 44

    with tile.TileContext(nc) as tc, ExitStack() as ctx:
        cpool = ctx.enter_context(tc.tile_pool(name="consts", bufs=1))
        inpool = ctx.enter_context(tc.tile_pool(name="inp", bufs=inpool_bufs))
        tpsum = ctx.enter_context(tc.tile_pool(name="tpsum", bufs=2, space="PSUM"))
        scr = ctx.enter_context(tc.tile_pool(name="scratch", bufs=1))
        wide = ctx.enter_context(tc.tile_pool(name="wide", bufs=2))
        apool = ctx.enter_context(tc.tile_pool(name="amask", bufs=a_bufs))
        maskp = ctx.enter_context(tc.tile_pool(name="masks", bufs=mask_bufs))
        mmpsum = ctx.enter_context(
            tc.tile_pool(name="mmpsum", bufs=mm_bufs, space="PSUM")
        )
        histp = ctx.enter_context(tc.tile_pool(name="hist", bufs=hist_bufs))

        # ---- constants ----
        ident = cpool.tile([128, 128], f32, tag="ident")
        make_identity(nc, ident[:])
        # iota_rep[p, j*AK + x] = j  (A-mask compare pattern, j in [0,QW))
        iota_rep_i = cpool.tile([128, QW * ak], i32, tag="iota_rep_i")
        nc.gpsimd.iota(
            iota_rep_i[:], pattern=[[1, QW], [0, ak]], base=0, channel_multiplier=0
        )
        iota_rep = cpool.tile([128, QW * ak], bf16, tag="iota_rep")
        nc.vector.tensor_copy(iota_rep[:], iota_rep_i[:])
        iota_rep_v = iota_rep[:].rearrange("p (j x) -> p j x", x=ak)
        # iota128[p, s] = s (C-mask compare pattern)
        iota128_i = cpool.tile([128, SW], i32, tag="iota128_i")
        nc.gpsimd.iota(iota128_i[:], pattern=[[1, SW]], base=0, channel_multiplier=0)
        iota128 = cpool.tile([128, SW], bf16, tag="iota128")
        nc.vector.tensor_copy(iota128[:], iota128_i[:])

        def wtt(name, out, a, b, op):
            eng = nc.gpsimd if name in wide_pool else nc.vector
            eng.tensor_tensor(out, a, b, op)

        for t in range(nt):
            rs = t * RT
            mzt = inpool.tile([128, P], f32, tag="mz")
            nc.sync.dma_start(mzt[:], mz_d[rs : rs + RT, :])
            itt = inpool.tile([128, P], f32, tag="it")
            nc.sync.dma_start(itt[:], it_d[rs : rs + RT, :])

            # ---- transpose to peak-major ----
            uT = scr.tile([128, P], f32, tag="uT")  # mz-10, peak-major
            sqT = wide.tile([128, P], f32, tag="sqT")  # sqrt(intensity)
            for c in range(NCHUNK):
                cs = slice(c * 128, (c + 1) * 128)
                pz = tpsum.tile([128, 128], f32, tag="tp")
                nc.tensor.transpose(pz[:], mzt[:, cs], ident[:])
                if u_act:
                    nc.scalar.activation(
                        uT[:, cs],
                        pz[:],
                        mybir.ActivationFunctionType.Copy,
                        bias=-10.0,
                        scale=1.0,
                    )
                else:
                    nc.vector.tensor_scalar(
                        uT[:, cs], pz[:], 10.0, None, mybir.AluOpType.subtract
                    )
                pz2 = tpsum.tile([128, 128], f32, tag="tp")
                nc.tensor.transpose(pz2[:], itt[:, cs], ident[:])
                nc.scalar.sqrt(sqT[:, cs], pz2[:])

            # ---- wide math: exact bin = floor(u / 0.1f) ----
            # q = RN(u/0.1f) via qhi = RN(10u), exact residual r = 10u - qhi
            # (8u, 2u exact), correction  q = RN(qhi + RN(r - EPS_D*qhi)).
            # Split into column halves so the first masks start sooner.
            qhi = scr.tile([128, P], f32, tag="qhi")
            t8 = scr.tile([128, P], f32, tag="t8")
            d = scr.tile([128, P], f32, tag="d")
            r_ = scr.tile([128, P], f32, tag="r_")
            s1 = scr.tile([128, P], f32, tag="s1")
            qq = scr.tile([128, P], f32, tag="qq")
            b0 = scr.tile([128, P], i32, tag="b0")
            f0 = scr.tile([128, P], f32, tag="f0")
            cond = scr.tile([128, P], f32, tag="cond")
            ff = scr.tile([128, P], f32, tag="ff")
            bini = scr.tile([128, P], i32, tag="bini")
            qi = scr.tile([128, P], i32, tag="qi")
            si = scr.tile([128, P], i32, tag="si")
            q_bf = wide.tile([128, P], bf16, tag="q_bf")
            s_f = wide.tile([128, P], f32, tag="s_f")
            wsplit = wide_split if t > 0 else first_split
            for h in range(wsplit):
                hs = slice(h * (P // wsplit), (h + 1) * (P // wsplit))
                nc.vector.tensor_scalar(
                    qhi[:, hs], uT[:, hs], 10.0, None, mybir.AluOpType.mult
                )
                nc.vector.tensor_scalar(
                    t8[:, hs], uT[:, hs], 8.0, None, mybir.AluOpType.mult
                )
                wtt("d", d[:, hs], t8[:, hs], qhi[:, hs], mybir.AluOpType.subtract)
                nc.vector.scalar_tensor_tensor(
                    r_[:, hs],
                    in0=uT[:, hs],
                    scalar=2.0,
                    in1=d[:, hs],
                    op0=mybir.AluOpType.mult,
                    op1=mybir.AluOpType.add,
                )
                nc.vector.scalar_tensor_tensor(
                    s1[:, hs],
                    in0=qhi[:, hs],
                    scalar=-EPS_D,
                    in1=r_[:, hs],
                    op0=mybir.AluOpType.mult,
                    op1=mybir.AluOpType.add,
                )
                wtt("q", qq[:, hs], qhi[:, hs], s1[:, hs], mybir.AluOpType.add)
                # robust floor (f32->i32 convert is round-to-nearest on HW)
                (nc.scalar.copy if "b0" in copies_act else
                 nc.vector.tensor_copy)(b0[:, hs], qq[:, hs])
                (nc.scalar.copy if "f0" in copies_act else
                 nc.vector.tensor_copy)(f0[:, hs], b0[:, hs])
                wtt(
                    "cond", cond[:, hs], f0[:, hs], qq[:, hs],
                    mybir.AluOpType.is_gt,
                )
                wtt(
                    "ff", ff[:, hs], f0[:, hs], cond[:, hs],
                    mybir.AluOpType.subtract,
                )
                (nc.scalar.copy if "bini" in copies_act else
                 nc.vector.tensor_copy)(bini[:, hs], ff[:, hs])  # ff integral
                # q/s split (power-of-2: exact bit ops; bitVec ops cannot
                # cast, so convert via separate copies)
                nc.vector.tensor_scalar(
                    qi[:, hs], bini[:, hs], 7, None,
                    mybir.AluOpType.arith_shift_right,
                )
                nc.vector.tensor_scalar(
                    si[:, hs], bini[:, hs], 127, None,
                    mybir.AluOpType.bitwise_and,
                )
                (nc.scalar.copy if "q_bf" in copies_act else
                 nc.vector.tensor_copy)(q_bf[:, hs], qi[:, hs])
                (nc.scalar.copy if "s_f" in copies_act else
                 nc.vector.tensor_copy)(s_f[:, hs], si[:, hs])
            use_act = bool(c_pat[2])
            if use_act:
                # -sqrt(it), the ACT C-mask Relu scale operand
                nvT = wide.tile([128, P], f32, tag="nvT")
                nc.vector.tensor_scalar(
                    nvT[:], sqT[:], -1.0, None, mybir.AluOpType.mult
                )

            # ---- masks + matmuls, by 16-row A-groups / 8-row PSUM groups ----
            for g in range(RT // ak):
                # batched A-masks for rows [g*AK, (g+1)*AK), all 8 chunks
                amasks = []
                for c in range(NCHUNK):
                    a = apool.tile([128, QW * ak], bf16, tag="A")
                    av = a[:].rearrange("p (j x) -> p j x", x=ak)
                    col0 = c * 128 + g * ak
                    in1 = (
                        q_bf[:, col0 : col0 + AK]
                        .rearrange("p (one x) -> p one x", one=1)
                        .to_broadcast([128, QW, ak])
                    )
                    nc.vector.tensor_tensor(
                        av, iota_rep_v, in1, mybir.AluOpType.is_equal
                    )
                    amasks.append(av)
                for sub in range(ak // RG):
                    pr = mmpsum.tile([QW, RG * SW], f32, tag="mm")
                    prv = pr[:].rearrange("q (r s) -> q r s", s=SW)
                    for r8 in range(RG):
                        x = sub * RG + r8  # row within A-group
                        r = g * ak + x  # row within row-tile
                        for c in range(NCHUNK):
                            col = c * 128 + r
                            idx = r * NCHUNK + c
                            cm = maskp.tile([128, SW], bf16, tag="C")
                            # first row-group of the run: DVE is busy with
                            # the wide chain, so route C-masks to Pool/ACT
                            pat = (
                                first_pat
                                if (first_pat and t == 0 and g == 0
                                    and sub == 0)
                                else c_pat
                            )
                            sel = idx % pat[0]
                            if sel in pat[2]:
                                # ACT: t = |s - iota|; cm = relu(v - v*t)
                                at = maskp.tile([128, SW], bf16, tag="At")
                                nc.scalar.activation(
                                    at[:],
                                    iota128[:],
                                    mybir.ActivationFunctionType.Abs,
                                    bias=s_f[:, col : col + 1],
                                    scale=-1.0,
                                )
                                nc.scalar.activation(
                                    cm[:],
                                    at[:],
                                    mybir.ActivationFunctionType.Relu,
                                    bias=sqT[:, col : col + 1],
                                    scale=nvT[:, col : col + 1],
                                )
                            else:
                                eng = (
                                    nc.gpsimd
                                    if sel in pat[1]
                                    else nc.vector
                                )
                                eng.tensor_scalar(
                                    cm[:],
                                    iota128[:],
                                    s_f[:, col : col + 1],
                                    sqT[:, col : col + 1],
                                    mybir.AluOpType.is_equal,
                                    mybir.AluOpType.mult,
                                )
                            nc.tensor.matmul(
                                prv[:, r8, :],
                                lhsT=amasks[c][:, :, x],
                                rhs=cm[:],
                                start=(c == 0),
                                stop=(c == NCHUNK - 1),
                            )
                    # ---- stage PSUM -> SBUF (ACT), DMA out ----
                    hist_sb = histp.tile([QW, RG * SW], f32, tag="hist")
                    nc.scalar.copy(hist_sb[:], pr[:])
                    g_abs = (rs + g * AK + sub * RG) // RG
                    hv = hist_sb[:].rearrange("q (r s) -> q r s", s=SW)
                    nc.sync.dma_start(out_main[g_abs], hv[0 : QW - 1])
                    tail = hist_sb[QW - 1 : QW, :].rearrange(
                        "one (r s) -> one r s", s=SW
                    )[:, :, 0:tail_w]
                    nc.sync.dma_start(out_tail[g_abs], tail)

    nc.compile()
    return nc


_CACHE: dict[int, object] = {}


def _get_program(rows_per_core: int):
    if rows_per_core not in _CACHE:
        _CACHE[rows_per_core] = build_program(rows_per_core)
    return _CACHE[rows_per_core]


def kernel(mz: np.ndarray, intensities: np.ndarray, trace: bool = False):
    mz = np.ascontiguousarray(np.asarray(mz, dtype=np.float32))
    intensities = np.ascontiguousarray(np.asarray(intensities, dtype=np.float32))
    bb = mz.shape[0]
    rows_per_core = bb // N_CORES
    nc = _get_program(rows_per_core)
    in_maps = []
    for i in range(N_CORES):
        sl = slice(i * rows_per_core, (i + 1) * rows_per_core)
        in_maps.append({"mz": mz[sl], "intensities": intensities[sl]})
    try:
        res = run_bass_kernel_spmd(
            nc, in_maps, core_ids=list(range(N_CORES)), trace=trace
        )
    except ModuleNotFoundError:
        res = run_bass_kernel_spmd(
            nc, in_maps, core_ids=list(range(N_CORES)), trace=False
        )
    out = np.concatenate([res.results[i]["out"] for i in range(N_CORES)], axis=0)
    if trace:
        kernel.last_exec_time_ns = res.exec_time_ns
        kernel.last_results = res
    return out


kernel.last_exec_time_ns = None
